# revision 1
# baseline (speedup 1.0000x reference)
"""Trainium2 Bass kernel for nn_CustomLoss_6330781795106.

Math (derived from the reference):
  p = softmax(y_pred, axis=1) clipped to [1e-7, 1]; th = 1/C
  per row i (label l_i, argmax a_i):
    py_i  = clip(exp(y[i,l_i]) / s_i, 1e-7, 1),  s_i = sum_j exp(y[i,j])
    nl_i  = (py_i - 1) * log(clip(1 - py_i, 1e-7, 1))
    ce2_i = a_i * log(py_i)
    mask_i = [second_largest(p_i) < th]       (== cnt_i < 2; max(p_i) >= th always)
    pyD_i = mask_i * max(p_i)
  loss = sum(nl)/B + 0.01 * ( -prod(1 + pyD^2) * sum(ce2) )

Data-parallel over 8 cores (1024 rows each). Per core, the [1024, 4096] scan
produces five per-row scalars: exp-sum s (ACT Exp in-place with fused accum),
exp(max) and exp(2nd max) and argmax index (DVE max8 + max_index on the exp'd
tile; exp is strictly monotone so order/indices match y's), and the label
logit (one indirect-DMA gather). A tiny [128, 8] epilogue reduces to
per-partition partial sums; the host sums those and combines the 8 cores.

Raw Bass (no Tile): this walrus build encodes at most ONE sync-wait per
instruction, so cross-engine deps are expressed as standalone wait_ge
sequencer ops with a static 4-semaphore schedule:
  SP:   4 double-tile loads (no waits) ... out store at the end
  ACT:  8 in-place Exp+accum (each waits its load), then epilogue Ln/Exp
  DVE:  8 max8/max_index + stat copies (each waits its Exp), then epilogue
  POOL: offs load + label gather (SWDGE)
"""

import numpy as np

try:
    import concourse.bass as bass
except ImportError:  # pragma: no cover
    import sys

    sys.path.insert(0, "/opt/trn_rl_repo")
    import concourse.bass as bass

import concourse.mybir as mybir
from concourse.bass_utils import run_bass_kernel_spmd

B = 8192          # global batch
C = 4096          # classes
NCORES = 8
R = B // NCORES   # rows per core (1024)
P = 128           # partitions
T = R // P        # row-tiles per core (8)
TH = 1.0 / C
F32 = mybir.dt.float32
U32 = mybir.dt.uint32
AF = mybir.ActivationFunctionType
ALU = mybir.AluOpType
X = mybir.AxisListType.X


def _build(R=R, debug_out=False):
    T = R // P
    nc = bass.Bass("TRN2", debug=False)
    y_d = nc.dram_tensor("y", [R, C], F32, kind="ExternalInput")
    off_d = nc.dram_tensor("off", [P, T], U32, kind="ExternalInput")
    rb_d = nc.dram_tensor("rb", [P, 1], F32, kind="ExternalInput")
    out_d = nc.dram_tensor("out", [P, 4], F32, kind="ExternalOutput")
    dbg_d = None
    if debug_out:
        dbg_d = nc.dram_tensor("dbg", [P, 8 * T], F32, kind="ExternalOutput")

    from contextlib import ExitStack
    with ExitStack() as ctx:
        def sb(name, shape, dt=F32):
            return ctx.enter_context(nc.sbuf_tensor(name, shape, dt))

        NCH = C // 128               # chunks per row (32)
        yt = sb("yt", [P, T * C])    # whole shard resident: 128 KiB/partition
        s8 = sb("s8", [P, T]); a8 = sb("a8", [P, T]); ylab = sb("ylab", [P, T])
        offs = sb("offs", [P, T], U32)
        r32 = sb("r32", [P, NCH * T])        # chunk maxes (exp domain)
        cm8 = sb("cm8", [P, 8 * T])          # top-8 chunk maxes per tile
        ci8 = sb("ci8", [P, 8 * T], U32)     # winning-chunk indices
        cif = sb("cif", [P, T])              # winning chunk idx as f32
        offw = sb("offw", [P, T], mybir.dt.int32)  # gather offsets
        offwf = sb("offwf", [P, T])
        wraw = sb("wraw", [P, T * 128])      # gathered winner chunks
        wm8 = sb("wm8", [P, 8 * T]); wi8 = sb("wi8", [P, 8 * T], U32)
        u = sb("u", [P, T]); rs = sb("rs", [P, T]); pyr = sb("pyr", [P, T])
        py = sb("py", [P, T]); t1 = sb("t1", [P, T]); t1c = sb("t1c", [P, T])
        l1 = sb("l1", [P, T]); nlp = sb("nlp", [P, T]); lp = sb("lp", [P, T])
        ce2 = sb("ce2", [P, T]); q2 = sb("q2", [P, T]); msk = sb("msk", [P, T])
        mp = sb("mp", [P, T]); pyD = sb("pyD", [P, T]); sq = sb("sq", [P, T])
        lw = sb("lw", [P, T]); v8t = sb("v8t", [P, T]); wif = sb("wif", [P, T])
        outsb = sb("outsb", [P, 4])
        rbp = sb("rbp", [P, 1])              # p*C per partition (input)
        # strided [P, T] views (step 8) over the per-tile top-8 outputs
        m8 = wm8[:].rearrange("p (t e) -> p t e", e=8)[:, :, 0]   # exp(max)
        c2v = cm8[:].rearrange("p (t e) -> p t e", e=8)[:, :, 1]  # 2nd chunk max
        w2v = wm8[:].rearrange("p (t e) -> p t e", e=8)[:, :, 1]  # 2nd in winner
        ciu = ci8[:].rearrange("p (t e) -> p t e", e=8)[:, :, 0]
        wiu = wi8[:].rearrange("p (t e) -> p t e", e=8)[:, :, 0]

        sem_y = ctx.enter_context(nc.semaphore("sem_y"))      # HWDGE out store
        # one sem per tile load: the 8 HWDGE queues complete out of order,
        # so a single counting sem cannot tell WHICH tiles have landed
        sem_t = [ctx.enter_context(nc.semaphore(f"sem_t{t}")) for t in range(T)]
        sem_sw = ctx.enter_context(nc.semaphore("sem_sw"))    # SWDGE offs+gather
        sem_act = ctx.enter_context(nc.semaphore("sem_act"))  # ACT progress
        sem_dve = ctx.enter_context(nc.semaphore("sem_dve"))  # DVE progress
        sem_gv = ctx.enter_context(nc.semaphore("sem_gv"))    # gather offsets ready
        block = ctx.enter_context(nc.Block())

        # static DVE-op counts (sem_dve values) for cross-engine waits
        N_T1C = 8 * T + 9    # ...through t1c
        N_SQ = 8 * T + 14    # ...through sq
        N_DVE_OPS = 8 * T + 20
        # ACT-op counts (sem_act values)
        A_U = 2 * T + 1      # u = exp(ylab), after T exps + T winner exps
        A_LP = 2 * T + 3     # l1, lp
        A_LW = 2 * T + 4     # lw

        @block.sync
        def _(sp):
            # one DMA per row-tile: the 8 HWDGE queues drain round-robin, so
            # tile t completes ~(t+1)/8 of the way through the total load
            # time and the ACT/DVE pipeline ramps up almost immediately
            for t in range(T):
                sp.dma_start(yt[:, t * C:(t + 1) * C],
                             y_d[t * P:(t + 1) * P, :]).then_inc(sem_t[t], 16)
            sp.wait_ge(sem_dve, N_DVE_OPS)
            sp.dma_start(out_d[:, :], outsb[:]).then_inc(sem_y, 16)
            ndma = 1
            if dbg_d is not None:
                dbg_groups = [s8[:], m8, v8t[:], a8[:], ylab[:], u[:], py[:],
                              lw[:]]
                with nc.allow_non_contiguous_dma(reason="debug-only dumps"):
                    for gi, g in enumerate(dbg_groups):
                        sp.dma_start(dbg_d[:, gi * T:(gi + 1) * T],
                                     g).then_inc(sem_y, 16)
                ndma += 8
            sp.wait_ge(sem_y, 16 * ndma)  # drain the stores

        @block.gpsimd
        def _(pl):
            pl.dma_start(offs[:], off_d[:, :]).then_inc(sem_sw, 16)
            pl.dma_start(rbp[:], rb_d[:, :]).then_inc(sem_sw, 16)
            pl.wait_ge(sem_sw, 32)
            # flat element indices into [R, C] (axis=1 => coefficient 1)
            pl.indirect_dma_start(
                out=ylab[:], out_offset=None,
                in_=y_d[:, :],
                in_offset=bass.IndirectOffsetOnAxis(ap=offs[:], axis=1),
            ).then_inc(sem_sw, 16)
            # winner-chunk gathers: 128 contiguous elements per row, offsets
            # computed on DVE from the winning chunk index
            for t in range(T):
                pl.wait_ge(sem_gv, t + 1)
                pl.indirect_dma_start(
                    out=wraw[:, t * 128:(t + 1) * 128], out_offset=None,
                    in_=y_d[:, :],
                    in_offset=bass.IndirectOffsetOnAxis(
                        ap=offw[:, t:t + 1], axis=1),
                    element_offset=t * P * C,
                ).then_inc(sem_sw, 16)

        @block.scalar
        def _(act):
            # in-place exp of each row-tile + fused row sum. No
            # max-subtraction: logits are standard normals, exp() is safe in
            # f32 and softmax is shift-invariant.
            for t in range(T):
                act.wait_ge(sem_t[t], 16)
                act.activation(out=yt[:, t * C:(t + 1) * C],
                               in_=yt[:, t * C:(t + 1) * C], func=AF.Exp,
                               accum_out=s8[:, t:t + 1]).then_inc(sem_act, 1)
            for t in range(T):
                act.wait_ge(sem_sw, 48 + 16 * (t + 1))
                act.activation(out=wraw[:, t * 128:(t + 1) * 128],
                               in_=wraw[:, t * 128:(t + 1) * 128],
                               func=AF.Exp).then_inc(sem_act, 1)
            act.activation(out=u[:], in_=ylab[:],
                           func=AF.Exp).then_inc(sem_act, 1)        # A_U
            act.wait_ge(sem_dve, N_T1C)
            act.activation(out=l1[:], in_=t1c[:],
                           func=AF.Ln).then_inc(sem_act, 1)
            act.activation(out=lp[:], in_=py[:],
                           func=AF.Ln).then_inc(sem_act, 1)         # A_LP
            act.wait_ge(sem_dve, N_SQ)
            act.activation(out=lw[:], in_=sq[:], func=AF.Ln,
                           bias=1.0).then_inc(sem_act, 1)           # A_LW

        @block.vector
        def _(dve):
            # The DVE pipeline is deep: same-engine RAW needs an explicit
            # self-semaphore (then_inc + wait) between dependent ops. dprog
            # tracks completed-DVE-op count; dwait() orders against all
            # prior DVE ops (the stream is essentially a dependence chain).
            state = {"n": 0}

            def step(inst):
                inst.then_inc(sem_dve, 1)
                state["n"] += 1

            def dwait():
                dve.wait_ge(sem_dve, state["n"])

            dve.wait_ge(sem_sw, 32)                 # rbp loaded
            for t in range(T):
                dve.wait_ge(sem_act, t + 1)
                tile = yt[:, t * C:(t + 1) * C]
                t3 = tile.rearrange("p (a c) -> p a c", c=128)
                sl = slice(8 * t, 8 * (t + 1))
                rsl = r32[:, NCH * t:NCH * (t + 1)]
                # level 1: 32 chunk maxes (the only full-row DVE pass)
                step(dve.tensor_reduce(rsl, t3, axis=X, op=ALU.max))
                dwait()
                step(dve.max(out=cm8[:, sl], in_=rsl))
                dwait()
                step(dve.max_index(out=ci8[:, sl], in_max=cm8[:, sl],
                                   in_values=rsl))
                dwait()
                step(dve.tensor_copy(cif[:, t:t + 1],
                                     ci8[:, 8 * t:8 * t + 1].bitcast(
                                         mybir.dt.int32)))
                dwait()
                step(dve.tensor_scalar_mul(offwf[:, t:t + 1],
                                           cif[:, t:t + 1], 128.0))
                dwait()
                step(dve.tensor_add(offwf[:, t:t + 1], offwf[:, t:t + 1],
                                    rbp[:]))
                dwait()
                # only sem_gv here (one sem update per instruction); not
                # counted in the DVE chain — only POOL consumes offw
                dve.tensor_copy(offw[:, t:t + 1],
                                offwf[:, t:t + 1]).then_inc(sem_gv, 1)
            # level 2: max8 + index inside each row's winning 128-wide chunk
            for t in range(T):
                dve.wait_ge(sem_act, T + t + 1)     # winner chunk exp'd
                wsl = wraw[:, t * 128:(t + 1) * 128]
                sl = slice(8 * t, 8 * (t + 1))
                step(dve.max(out=wm8[:, sl], in_=wsl))
                dwait()
                step(dve.max_index(out=wi8[:, sl], in_max=wm8[:, sl],
                                   in_values=wsl))
            # epilogue: a linear dependence chain of tiny [P, T] ops
            step(dve.reciprocal(rs[:], s8[:]))
            dwait()
            step(dve.tensor_copy(wif[:], wiu.bitcast(mybir.dt.int32)))
            dwait()
            step(dve.tensor_scalar_mul(a8[:], cif[:], 128.0))
            dwait()
            step(dve.tensor_add(a8[:], a8[:], wif[:]))
            dwait()
            step(dve.tensor_tensor(out=v8t[:], in0=c2v, in1=w2v,
                                   op=ALU.max))
            dve.wait_ge(sem_act, A_U)               # u = exp(ylab) ready
            dwait()
            step(dve.tensor_mul(pyr[:], u[:], rs[:]))
            dwait()
            step(dve.tensor_scalar(py[:], pyr[:], 1e-7, 1.0, op0=ALU.max,
                                   op1=ALU.min))
            dwait()
            step(dve.tensor_scalar(t1[:], py[:], -1.0, 1.0, op0=ALU.mult,
                                   op1=ALU.add))    # 1 - py
            dwait()
            step(dve.tensor_scalar_max(t1c[:], t1[:], 1e-7))
            step(dve.tensor_mul(q2[:], v8t[:], rs[:]))  # second-largest prob
            dwait()
            step(dve.tensor_scalar(msk[:], q2[:], TH, None, op0=ALU.is_lt))
            step(dve.tensor_mul(mp[:], m8, rs[:]))  # max prob
            dwait()
            step(dve.tensor_mul(pyD[:], msk[:], mp[:]))
            dwait()
            step(dve.tensor_mul(sq[:], pyD[:], pyD[:]))
            dve.wait_ge(sem_act, A_LP)              # l1, lp ready
            step(dve.tensor_mul(nlp[:], t1[:], l1[:]))  # host negates
            step(dve.tensor_mul(ce2[:], a8[:], lp[:]))
            dwait()
            step(dve.tensor_reduce(outsb[:, 0:1], nlp[:], axis=X, op=ALU.add))
            step(dve.tensor_reduce(outsb[:, 1:2], ce2[:], axis=X, op=ALU.add))
            dve.wait_ge(sem_act, A_LW)              # lw ready
            step(dve.tensor_reduce(outsb[:, 2:3], lw[:], axis=X, op=ALU.add))
            dwait()
            step(dve.tensor_reduce(outsb[:, 3:4], a8[:], axis=X, op=ALU.add))
            assert state["n"] == N_DVE_OPS, state["n"]
    return nc


def _in_maps(y, lab):
    maps = []
    for c in range(NCORES):
        ys = np.ascontiguousarray(y[c * R:(c + 1) * R])
        labs = lab[c * R:(c + 1) * R].astype(np.int64)
        r = np.arange(R, dtype=np.int64)
        flat = (r * C + labs).astype(np.uint32)
        off = np.ascontiguousarray(flat.reshape(T, P).T)  # [P, T]
        rb = (np.arange(P, dtype=np.float32) * C).reshape(P, 1)
        maps.append({"y": ys, "off": off, "rb": rb})
    return maps


def _combine(results):
    nlp_sum = 0.0
    ce2_sum = 0.0
    lw_sum = 0.0
    for c in range(NCORES):
        o = np.asarray(results[c]["out"], dtype=np.float64)
        nlp_sum += o[:, 0].sum()
        ce2_sum += o[:, 1].sum()
        lw_sum += o[:, 2].sum()
    nl = -nlp_sum / float(B)
    pl = -np.exp(lw_sum) * ce2_sum
    return np.array([nl + 0.01 * pl], dtype=np.float32)


def kernel(y_pred, y_true2):
    y = np.ascontiguousarray(np.asarray(y_pred, dtype=np.float32))
    lab = np.asarray(y_true2).astype(np.int64)
    assert y.shape == (B, C) and lab.shape == (B,)
    nc = _build()
    res = run_bass_kernel_spmd(nc, _in_maps(y, lab),
                               core_ids=list(range(NCORES))).results
    return _combine(res)



# revision 8
# speedup vs baseline: 1.0931x; 1.0931x over previous
"""Trainium2 Bass kernel for nn_CustomLoss_6330781795106.

Math (derived from the reference):
  p = softmax(y_pred, axis=1); th = 1/C
  per row i (label l_i, argmax a_i, L_i = ln(sum_j exp(y_ij))):
    nl_i  = (py_i - 1) * ln(1 - py_i),   py_i = exp(y[i,l_i] - L_i)
    ce2_i = a_i * (y[i,l_i] - L_i)                       (= a_i * ln(py_i))
    mask_i = [v2_i < L_i + ln(th)]    (v2 = 2nd-largest logit; cnt_i < 2)
    pyD_i = mask_i * exp(vmax_i - L_i)
  loss = sum(nl)/B + 0.01 * ( -prod(1 + pyD^2) * sum(ce2) )

Data-parallel over 8 cores (1024 rows each). Per core the [1024, 4096] shard
is loaded as 16 half-tiles [128, 2048] on one FIFO HWDGE ring, so pieces land
in order every ~3 us and compute pipelines behind the load:
  ACT:  exp of each raw half-tile into a scratch dump, fused accum -> exp-sum
        (the elementwise output is discarded; only the row sum is needed)
  DVE:  chunk-max tensor_reduce of each raw half-tile (exp is monotone, all
        max/argmax logic runs in the raw-logit domain), then per tile a
        max8 + max_index over the 32 chunk maxes
  GPSIMD: winner-chunk gather offsets (one int add: chunk-id + 32*partition)
        and the SWDGE gathers of the winning 128-wide chunks + label logits
  DVE:  max8 + max_index inside each gathered winner chunk, then a short
        batched [128, 8] epilogue in the log domain
All-reduce across cores happens on host (tiny [128,4] per-core outputs).

Raw Bass: one sem-wait per instruction; cross-engine deps are standalone
wait_ge ops against static per-engine op counts. Same-engine RAW on DVE
needs an explicit self-semaphore (deep pipeline), hence the dwait pattern.
"""

import numpy as np

try:
    import concourse.bass as bass
except ImportError:  # pragma: no cover
    import sys

    sys.path.insert(0, "/opt/trn_rl_repo")
    import concourse.bass as bass

import concourse.mybir as mybir
from concourse.bass_utils import run_bass_kernel_spmd

B = 8192          # global batch
C = 4096          # classes
NCORES = 8
R = B // NCORES   # rows per core (1024)
P = 128           # partitions
T = R // P        # row-tiles per core (8)
HC = C // 2       # half-tile columns (2048)
NCH = C // 128    # 128-wide chunks per row (32)
LTH = float(np.log(1.0 / C))
F32 = mybir.dt.float32
U32 = mybir.dt.uint32
I32 = mybir.dt.int32
AF = mybir.ActivationFunctionType
ALU = mybir.AluOpType
X = mybir.AxisListType.X

# sem_act checkpoints (static): 16 exps then L, py, l1, em, lw
A_EXPS = 2 * T
A_L = A_EXPS + 1
A_PY = A_EXPS + 2
A_L1 = A_EXPS + 3
A_EM = A_EXPS + 4
A_LW = A_EXPS + 5


def _build(debug_out=False):
    nc = bass.Bass("TRN2", debug=False)
    y_d = nc.dram_tensor("y", [R, C], F32, kind="ExternalInput")
    # aux: cols 0..T-1 = flat label indices (row*C + label), col T = 32*p
    aux_d = nc.dram_tensor("auxd", [P, T + 1], U32, kind="ExternalInput")
    out_d = nc.dram_tensor("out", [P, 4], F32, kind="ExternalOutput")
    dbg_d = None
    if debug_out:
        dbg_d = nc.dram_tensor("dbg", [P, 8 * T], F32, kind="ExternalOutput")

    # y viewed as [R*NCH, 128]: gather offsets count 128-wide chunks (coef)
    y_chunks = y_d[:, :].rearrange("r (a c) -> (r a) c", c=128)

    from contextlib import ExitStack
    with ExitStack() as ctx:
        def sb(name, shape, dt=F32):
            return ctx.enter_context(nc.sbuf_tensor(name, shape, dt))

        yt = sb("yt", [P, T * C])            # whole raw shard: 128 KiB/part
        es = sb("es", [P, HC])               # exp dump (output discarded)
        s2 = sb("s2", [P, 2 * T])            # exp-sums per half-tile
        s8 = sb("s8", [P, T])                # exp-sums per tile
        r32 = sb("r32", [P, NCH * T])        # raw chunk maxes
        cm8 = sb("cm8", [P, 8 * T])          # top-8 chunk maxes per tile
        ci8 = sb("ci8", [P, 8 * T], U32)     # their chunk indices
        offw = sb("offw", [P, T], U32)       # gather offsets (chunk units)
        wraw = sb("wraw", [P, T * 128])      # gathered winner chunks (raw)
        wm8 = sb("wm8", [P, 8 * T])          # top-8 within winner chunk
        wi8 = sb("wi8", [P, 8 * T], U32)     # their within-chunk indices
        aux = sb("aux", [P, T + 1], U32)
        ylab = sb("ylab", [P, T])            # gathered label logits
        # epilogue [P, T] scratch
        L = sb("L", [P, T]); d = sb("d", [P, T])
        py = sb("py", [P, T]); l1 = sb("l1", [P, T]); nl8 = sb("nl8", [P, T])
        cifs = sb("cifs", [P, T]); wif = sb("wif", [P, T])
        a8f = sb("a8f", [P, T]); ce2 = sb("ce2", [P, T]); v2 = sb("v2", [P, T])
        msk = sb("msk", [P, T]); vsub = sb("vsub", [P, T]); em = sb("em", [P, T])
        pyD = sb("pyD", [P, T]); sq = sb("sq", [P, T]); lw = sb("lw", [P, T])
        outsb = sb("outsb", [P, 4])

        # strided [P, T] views (step 8) into per-tile top-8 outputs
        c2v = cm8[:].rearrange("p (t e) -> p t e", e=8)[:, :, 1]  # 2nd chunk max
        m8v = wm8[:].rearrange("p (t e) -> p t e", e=8)[:, :, 0]  # row max
        w2v = wm8[:].rearrange("p (t e) -> p t e", e=8)[:, :, 1]  # 2nd in winner
        ciu = ci8[:].rearrange("p (t e) -> p t e", e=8)[:, :, 0]
        wiu = wi8[:].rearrange("p (t e) -> p t e", e=8)[:, :, 0]
        rbc = aux[:, T:T + 1]                                     # 32*p (u32)

        sem_h = [ctx.enter_context(nc.semaphore(f"sem_h{k}"))
                 for k in range(2 * T)]                          # half loads
        sem_g = [ctx.enter_context(nc.semaphore(f"sem_g{t}"))
                 for t in range(T)]                           # winner gathers
        sem_y = ctx.enter_context(nc.semaphore("sem_y"))      # out store
        sem_sw = ctx.enter_context(nc.semaphore("sem_sw"))    # SWDGE DMAs
        sem_act = ctx.enter_context(nc.semaphore("sem_act"))  # ACT progress
        sem_dve = ctx.enter_context(nc.semaphore("sem_dve"))  # DVE progress
        sem_gv = ctx.enter_context(nc.semaphore("sem_gv"))    # offw_t ready
        block = ctx.enter_context(nc.Block())

        # ---------------- DVE (built first: records op-count checkpoints)
        dve_n = {"n": 0}
        n_idx = [0] * T      # sem_dve count after max_index of tile t
        n_chk = {}           # named epilogue checkpoints

        @block.vector
        def _(dve):
            def step(inst):
                inst.then_inc(sem_dve, 1)
                dve_n["n"] += 1

            def dwait():
                dve.wait_ge(sem_dve, dve_n["n"])

            def wm_ops(t):
                # top-8 + indices inside tile t's gathered winner chunk
                dve.wait_ge(sem_g[t], 16)
                sl = slice(8 * t, 8 * (t + 1))
                wsl = wraw[:, t * 128:(t + 1) * 128]
                step(dve.max(out=wm8[:, sl], in_=wsl))
                dwait()
                step(dve.max_index(out=wi8[:, sl], in_max=wm8[:, sl],
                                   in_values=wsl))

            for t in range(T):
                for h in range(2):
                    dve.wait_ge(sem_h[2 * t + h], 16)
                    half = yt[:, t * C + h * HC:t * C + (h + 1) * HC]
                    h3 = half.rearrange("p (a c) -> p a c", c=128)
                    step(dve.tensor_reduce(
                        r32[:, NCH * t + 16 * h:NCH * t + 16 * (h + 1)],
                        h3, axis=X, op=ALU.max))
                dwait()
                sl = slice(8 * t, 8 * (t + 1))
                rsl = r32[:, NCH * t:NCH * (t + 1)]
                step(dve.max(out=cm8[:, sl], in_=rsl))
                dwait()
                step(dve.max_index(out=ci8[:, sl], in_max=cm8[:, sl],
                                   in_values=rsl))
                n_idx[t] = dve_n["n"]
                if t >= 1:
                    wm_ops(t - 1)
            wm_ops(T - 1)

            # ---- epilogue: batched [P, T] ops, log domain
            dve.wait_ge(sem_act, A_EXPS)            # all exp accums landed
            s2v = s2[:].rearrange("p (t e) -> p t e", e=2)
            step(dve.tensor_tensor(out=s8[:], in0=s2v[:, :, 0],
                                   in1=s2v[:, :, 1], op=ALU.add))
            n_chk["s8"] = dve_n["n"]
            dwait()      # orders reads of ci8/wi8 behind the last max_index
            step(dve.tensor_copy(cifs[:], ciu.bitcast(I32)))
            step(dve.tensor_copy(wif[:], wiu.bitcast(I32)))
            dwait()
            step(dve.scalar_tensor_tensor(out=a8f[:], in0=cifs[:],
                                          scalar=128.0, in1=wif[:],
                                          op0=ALU.mult, op1=ALU.add))
            dve.wait_ge(sem_sw, 32)                 # ylab gathered
            dve.wait_ge(sem_act, A_L)               # L ready
            step(dve.tensor_tensor(out=d[:], in0=ylab[:], in1=L[:],
                                   op=ALU.subtract))
            n_chk["d"] = dve_n["n"]
            step(dve.tensor_tensor(out=vsub[:], in0=m8v, in1=L[:],
                                   op=ALU.subtract))
            n_chk["vsub"] = dve_n["n"]
            step(dve.tensor_tensor(out=v2[:], in0=c2v, in1=w2v, op=ALU.max))
            dwait()
            step(dve.tensor_tensor(out=ce2[:], in0=d[:], in1=a8f[:],
                                   op=ALU.mult))
            step(dve.scalar_tensor_tensor(out=msk[:], in0=v2[:],
                                          scalar=LTH, in1=L[:],
                                          op0=ALU.subtract, op1=ALU.is_lt))
            dve.wait_ge(sem_act, A_EM)
            dwait()
            step(dve.tensor_tensor(out=pyD[:], in0=msk[:], in1=em[:],
                                   op=ALU.mult))
            dwait()
            step(dve.tensor_tensor(out=sq[:], in0=pyD[:], in1=pyD[:],
                                   op=ALU.mult))
            n_chk["sq"] = dve_n["n"]
            dve.wait_ge(sem_act, A_L1)
            step(dve.scalar_tensor_tensor(out=nl8[:], in0=py[:], scalar=1.0,
                                          in1=l1[:], op0=ALU.subtract,
                                          op1=ALU.mult))
            dwait()
            step(dve.tensor_reduce(outsb[:, 0:1], nl8[:], axis=X, op=ALU.add))
            step(dve.tensor_reduce(outsb[:, 1:2], ce2[:], axis=X, op=ALU.add))
            dve.wait_ge(sem_act, A_LW)
            step(dve.tensor_reduce(outsb[:, 2:3], lw[:], axis=X, op=ALU.add))
            step(dve.tensor_reduce(outsb[:, 3:4], a8f[:], axis=X, op=ALU.add))

        n_dve_total = dve_n["n"]

        # ---------------- GPSIMD: aux load, gathers, offset math
        @block.gpsimd
        def _(pl):
            pl.dma_start(aux[:], aux_d[:, :]).then_inc(sem_sw, 16)
            pl.wait_ge(sem_sw, 16)
            pl.indirect_dma_start(
                out=ylab[:], out_offset=None,
                in_=y_d[:, :],
                in_offset=bass.IndirectOffsetOnAxis(ap=aux[:, 0:T], axis=1),
            ).then_inc(sem_sw, 16)
            for t in range(T):
                pl.wait_ge(sem_dve, n_idx[t])
                # winner-chunk id -> global chunk id: + 32*p (+ t*4096 via
                # element_offset below, in raw-element units)
                pl.tensor_tensor(out=offw[:, t:t + 1],
                                 in0=ci8[:, 8 * t:8 * t + 1],
                                 in1=rbc,
                                 op=ALU.add).then_inc(sem_gv, 1)
                pl.wait_ge(sem_gv, t + 1)
                pl.indirect_dma_start(
                    out=wraw[:, t * 128:(t + 1) * 128], out_offset=None,
                    in_=y_chunks,
                    in_offset=bass.IndirectOffsetOnAxis(
                        ap=offw[:, t:t + 1], axis=0),
                    element_offset=t * P * C,
                ).then_inc(sem_g[t], 16)

        # ---------------- ACT: exp passes + log-domain epilogue
        @block.scalar
        def _(act):
            for k in range(2 * T):
                act.wait_ge(sem_h[k], 16)
                if k >= 1:
                    # WAW order on the shared exp-dump buffer
                    act.wait_ge(sem_act, k)
                t, h = divmod(k, 2)
                half = yt[:, t * C + h * HC:t * C + (h + 1) * HC]
                act.activation(out=es[:], in_=half, func=AF.Exp,
                               accum_out=s2[:, k:k + 1]).then_inc(sem_act, 1)
            act.wait_ge(sem_dve, n_chk["s8"])
            act.activation(out=L[:], in_=s8[:],
                           func=AF.Ln).then_inc(sem_act, 1)          # A_L
            act.wait_ge(sem_dve, n_chk["d"])
            act.activation(out=py[:], in_=d[:],
                           func=AF.Exp).then_inc(sem_act, 1)         # A_PY
            act.wait_ge(sem_act, A_PY)
            act.activation(out=l1[:], in_=py[:], func=AF.Ln,
                           bias=1.0, scale=-1.0).then_inc(sem_act, 1)  # A_L1
            act.wait_ge(sem_dve, n_chk["vsub"])
            act.activation(out=em[:], in_=vsub[:],
                           func=AF.Exp).then_inc(sem_act, 1)         # A_EM
            act.wait_ge(sem_dve, n_chk["sq"])
            act.activation(out=lw[:], in_=sq[:], func=AF.Ln,
                           bias=1.0).then_inc(sem_act, 1)            # A_LW

        # ---------------- SP: the 16 half-tile loads + final store
        @block.sync
        def _(sp):
            for t in range(T):
                for h in range(2):
                    sp.dma_start(
                        yt[:, t * C + h * HC:t * C + (h + 1) * HC],
                        y_d[t * P:(t + 1) * P, h * HC:(h + 1) * HC],
                    ).then_inc(sem_h[2 * t + h], 16)
            sp.wait_ge(sem_dve, n_dve_total)
            sp.dma_start(out_d[:, :], outsb[:]).then_inc(sem_y, 16)
            ndma = 1
            if dbg_d is not None:
                dbg_groups = [s8[:], m8v, v2[:], a8f[:], ylab[:], py[:],
                              lw[:], d[:]]
                with nc.allow_non_contiguous_dma(reason="debug-only dumps"):
                    for gi, g in enumerate(dbg_groups):
                        sp.dma_start(dbg_d[:, gi * T:(gi + 1) * T],
                                     g).then_inc(sem_y, 16)
                ndma += 8
            sp.wait_ge(sem_y, 16 * ndma)
    return nc


def _in_maps(y, lab):
    maps = []
    p = np.arange(P, dtype=np.uint32)
    for c in range(NCORES):
        ys = np.ascontiguousarray(y[c * R:(c + 1) * R])
        labs = lab[c * R:(c + 1) * R].astype(np.int64)
        r = np.arange(R, dtype=np.int64)
        flat = (r * C + labs).astype(np.uint32)
        aux = np.empty((P, T + 1), dtype=np.uint32)
        aux[:, :T] = flat.reshape(T, P).T  # aux[p, t] = (t*128+p)*C + lab
        aux[:, T] = p * NCH                # 32*p
        maps.append({"y": ys, "auxd": aux})
    return maps


def _combine(results):
    nl_sum = 0.0
    ce2_sum = 0.0
    lw_sum = 0.0
    for c in range(NCORES):
        o = np.asarray(results[c]["out"], dtype=np.float64)
        nl_sum += o[:, 0].sum()
        ce2_sum += o[:, 1].sum()
        lw_sum += o[:, 2].sum()
    nl = nl_sum / float(B)
    pl = -np.exp(lw_sum) * ce2_sum
    return np.array([nl + 0.01 * pl], dtype=np.float32)


def kernel(y_pred, y_true2):
    y = np.ascontiguousarray(np.asarray(y_pred, dtype=np.float32))
    lab = np.asarray(y_true2).astype(np.int64)
    assert y.shape == (B, C) and lab.shape == (B,)
    nc = _build()
    res = run_bass_kernel_spmd(nc, _in_maps(y, lab),
                               core_ids=list(range(NCORES))).results
    return _combine(res)
